# Initial kernel scaffold
#
"""Trainium2 Bass kernel for nn_Local_align: per-sample dynamic 3x3 conv.

  img = l2norm(vision, axis=C)                              [B,C,H,W]
  tf  = l2norm(text, axis=-1) @ Wt.T + bt                   [B,Nc,out_dim]
  w   = softmax(tf[..., :-1] grouped per (C, 3x3)), b = tf[..., -1]
  out[b] = conv2d_same(img[b], w[b]) + b                    [B,Nc,H,W]

Sharding: data-parallel over batch B=8, one image per NeuronCore.

Per-core structure:
  phase A (text -> conv weights): l2norm rows, PE-transpose, fp16 matmul
    against host-pretransposed Wt^T, softmax over the 9 taps (exp on ACT,
    group-sum + normalize on DVE), PE-transpose to per-tap [c, nc] fp16.
  normalize: per 512-px window: DMA, square (GPSIMD), channel-sumsq via
    all-ones f32r matmul (reduce+broadcast in one), sqrt (ACT), reciprocal
    (DVE), multiply into zero-padded fp16 image (DVE).
  conv: per window x nc-chunk: 18 accumulating fp16 matmuls (9 taps x 2
    c-chunks); the 22-wide nc remainder runs 4 windows concurrently via
    4x column tiling (tile_position). Bias add fused into the PSUM->SBUF
    copy (per-partition ACT/DVE add), then DMA out.
  Emission interleaves conv groups with normalize windows so PE overlaps
  the DMA/elementwise pipeline.
"""

import numpy as np

B = 8
C, H, W = 256, 128, 128
NC, KD = 150, 768
KK = 9  # 3x3 taps
OD = C * KK + 1  # 2305
HW = H * W  # 16384
PW = W + 2  # padded row width 130
PH = H + 2
WS = 512  # pixel window = 4 rows
NWIN = HW // WS  # 32
ROWS_PER_WIN = WS // W  # 4
NC0 = 128  # first nc chunk
NC1 = NC - NC0  # 22, handled via 4x col tiling
KCH = KD // 128  # 6
CCH = C // 128  # 2
GRP = 4  # windows per col-tiled nc1 group
# 9-aligned tf windows so per-window softmax group-sums don't cross windows;
# the +1 on the last window covers the bias column (od 2304)
TF_WINS = [(0, 504), (504, 504), (1008, 504), (1512, 504), (2016, 289)]


def _build_program(reps=1, ablate=frozenset()):
    import concourse.bacc as bacc
    import concourse.tile as tile
    from concourse import mybir

    f32 = mybir.dt.float32
    f16 = mybir.dt.float16

    nc = bacc.Bacc("TRN2", target_bir_lowering=False, debug=False)

    vis = nc.dram_tensor("vision", [C, HW], f32, kind="ExternalInput").ap()
    txt = nc.dram_tensor("text", [NC, KD], f32, kind="ExternalInput").ap()
    wtT = nc.dram_tensor("wtT", [KD, OD], f16, kind="ExternalInput").ap()
    btd = nc.dram_tensor("bt", [OD], f16, kind="ExternalInput").ap()
    out = nc.dram_tensor("out", [NC, HW], f32, kind="ExternalOutput").ap()
    aps = (vis, txt, wtT, btd, out)

    with tile.TileContext(nc) as tc:
        if reps == 1:
            _emit_iteration(nc, tc, mybir, aps, ablate)
        else:
            with tc.For_i(0, reps, 1):
                _emit_iteration(nc, tc, mybir, aps, ablate)

    nc.compile()
    return nc


def _emit_iteration(nc, tc, mybir, aps, ablate=frozenset()):
    from contextlib import ExitStack
    from concourse.masks import make_identity

    vis, txt, wtT, btd, out = aps
    f32 = mybir.dt.float32
    f32r = mybir.dt.float32r
    f16 = mybir.dt.float16
    MUL = mybir.AluOpType.mult
    DIV = mybir.AluOpType.divide
    AX = mybir.ActivationFunctionType
    X = mybir.AxisListType.X

    with ExitStack() as ctx:
        singles = ctx.enter_context(tc.tile_pool(name="singles", bufs=1))
        persist = ctx.enter_context(tc.tile_pool(name="persist", bufs=1))
        rawp = ctx.enter_context(tc.tile_pool(name="rawp", bufs=5))
        pssp = ctx.enter_context(tc.tile_pool(name="pss", bufs=2, space="PSUM"))

        # ---------------- constants ----------------
        ident32 = singles.tile([128, 128], f32)
        make_identity(nc, ident32)
        ident16 = singles.tile([128, 128], f16)
        make_identity(nc, ident16)
        ones32f = singles.tile([128, 128], f32)
        nc.gpsimd.memset(ones32f, 1.0)
        ones32 = singles.tile([128, 128], f32r)
        nc.scalar.copy(ones32, ones32f)
        ones16 = singles.tile([1, 128], f16)
        nc.gpsimd.memset(ones16, 1.0)
        bt16 = singles.tile([1, OD], f16)
        nc.sync.dma_start(out=bt16, in_=btd.unsqueeze(0))

        # ---------------- persistent tiles ----------------
        pads = [persist.tile([128, PH, PW], f16, name=f"pad{i}") for i in range(CCH)]
        wTs = [persist.tile([128, KK, NC], f16, name=f"wT{i}") for i in range(CCH)]
        t_hatT = persist.tile([128, KCH, NC], f16)
        tfs = [persist.tile([128, OD], f32, name=f"tf{i}") for i in range(2)]
        bias4 = persist.tile([128, 1], f32)  # nc1 bias replicated per col group

        for p in pads:
            nc.gpsimd.memset(p[:, 0, :], 0.0)
            nc.gpsimd.memset(p[:, PH - 1, :], 0.0)
            nc.gpsimd.memset(p[:, :, 0], 0.0)
            nc.gpsimd.memset(p[:, :, PW - 1], 0.0)

        # ---------------- phase A: text -> conv weights ----------------
        def emit_phase_a(between=None):
            with (
                tc.tile_pool(name="pa", bufs=1) as pa,
                tc.tile_pool(name="pawt", bufs=10) as pawt,
                tc.tile_pool(name="pst", bufs=2, space="PSUM") as pstp,
                tc.tile_pool(name="psw", bufs=2, space="PSUM") as pswp,
            ):
                t_sbs = []
                for i, (n0, cnt) in enumerate([(0, NC0), (NC0, NC1)]):
                    t_sb = pa.tile([128, KD], f32, name=f"t_sb{i}")
                    nc.sync.dma_start(out=t_sb[:cnt], in_=txt[n0:n0 + cnt])
                    t_sbs.append(t_sb)

                tsq = pa.tile([128, KD], f32)
                stat = pa.tile([128, 4], f32)
                for i, (n0, cnt) in enumerate([(0, NC0), (NC0, NC1)]):
                    t_sb = t_sbs[i]
                    nc.scalar.activation(
                        out=tsq[:cnt], in_=t_sb[:cnt], func=AX.Square,
                        accum_out=stat[:cnt, 0:1],
                    )
                    nc.scalar.sqrt(stat[:cnt, 1:2], stat[:cnt, 0:1])
                    nc.vector.reciprocal(stat[:cnt, 2:3], stat[:cnt, 1:2])
                    nc.vector.tensor_scalar_mul(t_sb[:cnt], t_sb[:cnt], stat[:cnt, 2:3])
                    for k in range(KCH):
                        pst = pstp.tile([128, 128], f32, name="pst", tag="pst")
                        nc.tensor.transpose(
                            pst[:, :cnt], t_sb[:cnt, k * 128:(k + 1) * 128],
                            ident32[:cnt, :cnt],
                        )
                        nc.scalar.copy(t_hatT[:, k, n0:n0 + cnt], pst[:, :cnt])

                if between is not None:
                    between()

                # tf = t_hat @ Wt.T + bt  (fp16 in, fp32 accum; Wt^T slices
                # streamed from DRAM, shared by both nc chunks), with the
                # softmax (exp, group-sum, normalize) pipelined per window
                chunks = [(0, 0, NC0), (1, NC0, NC1)]
                w16s = [
                    pa.tile([128, C * KK], f16, name=f"w16_{i}", tag=f"w16_{i}")
                    for i in range(2)
                ]

                def emit_wtrans(i, cc):
                    n0, cnt = [(0, NC0), (NC0, NC1)][i]
                    w16v = w16s[i].rearrange("p (c t) -> p c t", t=KK)
                    for tap in range(KK):
                        pst = pstp.tile([128, 128], f16, name="pst", tag="pst")
                        nc.tensor.transpose(
                            pst[:, :cnt],
                            w16v[:cnt, cc * 128:(cc + 1) * 128, tap],
                            ident16[:cnt, :cnt],
                        )
                        nc.scalar.copy(wTs[cc][:, tap, n0:n0 + cnt], pst[:, :cnt])

                for wi, (o0, ws) in enumerate(TF_WINS):
                    psws = [
                        pswp.tile([128, 512], f32, tag=f"psw{i}", name=f"psw{i}")
                        for i in range(2)
                    ]
                    for k in range(KCH):
                        wtsl = pawt.tile([128, 512], f16, tag="wtsl")
                        nc.sync.dma_start(
                            out=wtsl[:, :ws],
                            in_=wtT[k * 128:(k + 1) * 128, o0:o0 + ws],
                        )
                        for i, n0, cnt in chunks:
                            nc.tensor.matmul(
                                psws[i][:cnt, :ws],
                                t_hatT[:, k, n0:n0 + cnt],
                                wtsl[:, :ws],
                                start=(k == 0),
                                stop=False,
                            )
                    wse = ws - 1 if o0 + ws > C * KK else ws  # exp excl. bias col
                    ngrp = wse // KK
                    for i, n0, cnt in chunks:
                        nc.tensor.matmul(
                            psws[i][:cnt, :ws],
                            ones16[:1, :cnt],
                            bt16[:1, o0:o0 + ws],
                            start=False,
                            stop=True,
                        )
                        nc.scalar.copy(tfs[i][:cnt, o0:o0 + ws], psws[i][:cnt, :ws])
                        tfw = tfs[i][:cnt, o0:o0 + wse]
                        nc.scalar.activation(out=tfw, in_=tfw, func=AX.Exp)
                        tfv = tfw.rearrange("p (c t) -> p c t", t=KK)
                        ssum = pa.tile([128, C], f32, tag=f"ssum{i}", name="ssum")
                        nc.vector.reduce_sum(out=ssum[:cnt, :ngrp], in_=tfv, axis=X)
                        rsum = pa.tile([128, C], f32, tag=f"rsum{i}", name="rsum")
                        nc.vector.reciprocal(rsum[:cnt, :ngrp], ssum[:cnt, :ngrp])
                        nc.vector.tensor_tensor(
                            out=w16s[i][:cnt, o0:o0 + wse].rearrange(
                                "p (c t) -> p c t", t=KK
                            ),
                            in0=tfv,
                            in1=rsum[:cnt, :ngrp].unsqueeze(2).broadcast_to(
                                [cnt, ngrp, KK]
                            ),
                            op=MUL,
                        )
                        # weight transposes as soon as their c-block columns
                        # are done: cc=0 spans tf windows 0-2, cc=1 spans 2-4
                        if wi == 2:
                            emit_wtrans(i, 0)
                        elif wi == 4:
                            emit_wtrans(i, 1)

                # nc1 conv bias replicated to partitions 32j+m (m<22) for the
                # col-tiled epilogue
                for j in range(GRP):
                    nc.sync.dma_start(
                        out=bias4[32 * j:32 * j + NC1],
                        in_=tfs[1][:NC1, C * KK:C * KK + 1],
                    )

        # ---------------- vision normalize (software-pipelined emission) ----
        # front(w): DMA + squares + sumsq matmuls; mid(w): sqrt (lags 1);
        # back(w): reciprocal + multiplies into pad (lags 2). The lag keeps
        # each engine's FIFO free of head-of-line stalls on upstream engines.
        sqp = ctx.enter_context(tc.tile_pool(name="sqp", bufs=4))
        nrm = ctx.enter_context(tc.tile_pool(name="nrm", bufs=4))
        outp = ctx.enter_context(tc.tile_pool(name="outp", bufs=4))
        state = {}  # per-window tiles in flight

        def front(w):
            raws, ps = [], pssp.tile([128, WS], f32, tag="pss", name="ps")
            for cc in range(CCH):
                raw = rawp.tile([128, WS], f32, name=f"raw{cc}", tag=f"raw{cc}")
                if "visdma" not in ablate:
                    nc.sync.dma_start(
                        out=raw, in_=vis[cc * 128:(cc + 1) * 128, w * WS:(w + 1) * WS]
                    )
                raws.append(raw)
                if "norm" not in ablate:
                    sq = sqp.tile([128, WS], f32r, name=f"sq{cc}", tag=f"sq{cc}")
                    nc.scalar.square(sq, raw)
                    nc.tensor.matmul(
                        ps, ones32, sq, start=(cc == 0), stop=(cc == CCH - 1)
                    )
            state[w] = [raws, ps, None]

        def mid(w):
            if "norm" in ablate or "nomid" in ablate:
                return
            # inv = 1/sqrt(sumsq) in one ACT op, straight from PSUM
            inv = nrm.tile([128, WS], f32, tag="inv")
            nc.scalar.activation(out=inv, in_=state[w][1], func=AX.Abs_reciprocal_sqrt)
            state[w][2] = inv

        def back(w):
            raws, _, inv = state.pop(w)
            if "norm" in ablate or "nomid" in ablate:
                return
            r0 = w * ROWS_PER_WIN
            if "nomul" in ablate:
                return
            inv_v = inv.rearrange("p (r x) -> p r x", x=W)
            for cc in range(CCH):
                nc.vector.tensor_tensor(
                    out=pads[cc][:, 1 + r0:1 + r0 + ROWS_PER_WIN, 1:1 + W],
                    in0=raws[cc].rearrange("p (r x) -> p r x", x=W),
                    in1=inv_v,
                    op=MUL,
                )

        cursor = [0]  # next front window

        def step_norm():
            w = cursor[0]
            if w < NWIN:
                front(w)
            if w >= 1 and w - 1 < NWIN:
                mid(w - 1)
            if w >= 2 and w - 2 < NWIN:
                back(w - 2)
            cursor[0] += 1

        def ensure_norm(upto):
            # guarantee back(w) emitted for all w <= upto
            while cursor[0] - 3 < min(upto, NWIN - 1):
                step_norm()

        # prologue: get the normalize pipeline moving during phase A's stalls
        # (emitted after the latency-critical text-norm chain via `between`)
        def _prologue():
            for _ in range(6):
                step_norm()

        if "phasea" not in ablate:
            emit_phase_a(between=_prologue)
        else:
            _prologue()

        if "conv" in ablate:
            ensure_norm(NWIN - 1)
            return

        # ---------------- conv, interleaved with normalize ----------------
        with (
            tc.tile_pool(name="pso0", bufs=2, space="PSUM") as pso0,
            tc.tile_pool(name="pso1", bufs=1, space="PSUM") as pso1,
        ):
            contig = "contig" in ablate  # timing experiment: wrong values

            def rhs_ap(cc, r0, ty, tx):
                if contig:
                    flat = pads[cc].rearrange("p a b -> p (a b)")
                    return flat[:, r0 * PW:r0 * PW + WS]
                return pads[cc][:, r0 + ty:r0 + ty + ROWS_PER_WIN, tx:tx + W]

            ensure_norm(GRP)
            for g in range(NWIN // GRP):
                ensure_norm((g + 1) * GRP)
                # nc chunk 0 (128-wide), one psum tile per window
                for j in range(GRP):
                    w = g * GRP + j
                    r0 = w * ROWS_PER_WIN
                    ps = pso0.tile([128, WS], f32, tag="pso0")
                    mi = 0
                    for cc in range(CCH):
                        for ty in range(3):
                            for tx in range(3):
                                nc.tensor.matmul(
                                    ps,
                                    wTs[cc][:, ty * 3 + tx, 0:NC0],
                                    rhs_ap(cc, r0, ty, tx),
                                    start=(mi == 0),
                                    stop=(mi == CCH * KK - 1),
                                )
                                mi += 1
                    osb = outp.tile([128, WS], f32, tag="osb")
                    nc.scalar.add(osb, ps, add=tfs[0][:NC0, C * KK:C * KK + 1])
                    if "outdma" not in ablate:
                        nc.sync.dma_start(
                            out=out[0:NC0, w * WS:(w + 1) * WS], in_=osb
                        )
                # nc chunk 1 (22-wide): 4 windows concurrently via col tiling
                ps1 = pso1.tile([128, GRP, WS], f32, tag="pso1")
                noct = "noct" in ablate
                pbase = [0 if noct else 32 * j for j in range(GRP)]
                mi = 0
                for cc in range(CCH):
                    for ty in range(3):
                        for tx in range(3):
                            for j in range(GRP):
                                r0 = (g * GRP + j) * ROWS_PER_WIN
                                nc.tensor.matmul(
                                    ps1[pbase[j]:pbase[j] + NC1, j, :],
                                    wTs[cc][:, ty * 3 + tx, NC0:NC],
                                    rhs_ap(cc, r0, ty, tx),
                                    start=(mi == 0),
                                    stop=(mi == CCH * KK - 1),
                                    tile_position=(0, pbase[j]),
                                )
                            mi += 1
                osb1 = outp.tile([128, WS], f32, tag="osb1")
                for j in range(GRP):
                    nc.vector.tensor_scalar_add(
                        osb1[32 * j:32 * j + NC1],
                        ps1[32 * j:32 * j + NC1, j, :],
                        bias4[32 * j:32 * j + NC1],
                    )
                if "outdma" not in ablate:
                    for j in range(GRP):
                        w = g * GRP + j
                        nc.sync.dma_start(
                            out=out[NC0:NC, w * WS:(w + 1) * WS],
                            in_=osb1[32 * j:32 * j + NC1],
                        )


_NC_CACHE = {}


def _get_program(reps=1, ablate=frozenset()):
    ablate = frozenset(ablate)
    key = (reps, ablate)
    if key not in _NC_CACHE:
        _NC_CACHE[key] = _build_program(reps, ablate)
    return _NC_CACHE[key]


def _make_in_maps(vision, text, Wt, bt):
    wtT16 = np.ascontiguousarray(Wt.astype(np.float32).T).astype(np.float16)
    bt16 = bt.astype(np.float16)
    in_maps = []
    for b in range(B):
        in_maps.append({
            "vision": np.ascontiguousarray(vision[b].reshape(C, HW)),
            "text": np.ascontiguousarray(text[b, :, 0, :]),
            "wtT": wtT16,
            "bt": bt16,
        })
    return in_maps


def _run(vision, text, Wt, bt, trace=False):
    from concourse.bass_utils import run_bass_kernel_spmd

    nc = _get_program()
    in_maps = _make_in_maps(vision, text, Wt, bt)
    res = run_bass_kernel_spmd(nc, in_maps, list(range(B)), trace=trace)
    outs = np.stack([np.asarray(res.results[b]["out"]).reshape(NC, H, W) for b in range(B)])
    return outs, res


def kernel(vision, text, Wt, bt):
    outs, _ = _run(vision, text, Wt, bt, trace=False)
    return outs



# revision 6
# speedup vs baseline: 81.9439x; 81.9439x over previous
"""Trainium2 Bass kernel v2 for nn_Local_align: per-sample dynamic 3x3 conv.

  img = l2norm(vision, axis=C)                              [B,C,H,W]
  tf  = l2norm(text, axis=-1) @ Wt.T + bt                   [B,Nc,out_dim]
  w   = softmax(tf[..., :-1] grouped per (C, 3x3)), b = tf[..., -1]
  out[b] = conv2d_same(img[b], w[b]) + b                    [B,Nc,H,W]

Sharding: data-parallel over batch B=8, one image per NeuronCore.

v2 core idea: split w = 1/9 + dw (softmax sums to 1 over the 9 taps, and
the logits are small, so dw is tiny). The dw part of the conv runs in fp8
DoubleRow matmuls (both 128-channel k-tiles contracted per instruction);
fp8 quantization error only enters scaled by dw, so accuracy holds. The
1/9-uniform part is rank-1 in (nc, c): base = (1/9)*box3(s) with s the
channel sums of the normalized image (computed exactly from f32 raw sums
times inv).

base pipeline (standard-AP ops only; no fancy gathers):
  s rows -> DRAM scratch -> s_padT (image rows on partitions, x in free).
  vertical 3-sum: matmul against a slice of a constant tridiagonal
  matrix TRI[:, 4w:4w+4] -> ps_v [4, 130]; horizontal 3-sum: two small
  DVE adds -> base_line [4,128]; bounce via DRAM to bl [1,512]; inject
  into each conv PSUM with a K=1 ones-matmul appended to the tap chain.

Per-window pipeline:
  front: DMA raw f32r x2; squares -> fp8 (GPSIMD); channel-sumsq via one
    fp8 DoubleRow ones-matmul; channel-sum via f32r ones-matmuls.
  mid:   inv = S_I/sqrt(sumsq) (ACT, scale folded); s_stage = rawsum*inv
    (DVE, fp16, scaled S_I); DMA row slice to s_dram; reload to s_padT.
  back:  pad8 = raw*inv -> fp8 image, 2-ktile interleaved layout (DVE x2).
  bstage: TRI matmul + ACT evict + 2 DVE adds + DMA to bl_dram.
  conv:  per nc chunk {128, 22}: 9 DoubleRow matmuls + K=1 base inject;
    DVE eviction (ps + bias) -> fp16 x512; DMA out. Host divides by 512.
"""

import numpy as np

B = 8
C, H, W = 256, 128, 128
NC, KD = 150, 768
KK = 9  # 3x3 taps
OD = C * KK + 1  # 2305
HW = H * W  # 16384
PW = W + 2  # padded row width 130
PH = H + 2
PWA = 136  # pad8 row pitch: PH*PWA divisible by 16 (dual-fp8 ktile stride)
WS = 512  # pixel window = 4 rows
NWIN = HW // WS  # 32
ROWS_PER_WIN = WS // W  # 4
NC0 = 128  # first nc chunk
NC1 = NC - NC0  # 22
KCH = KD // 128  # 6
CCH = C // 128  # 2
S_W = 32.0  # dw scale into fp8
S_I = 16.0  # img scale into fp8
# 9-aligned tf windows so per-window softmax group-sums don't cross windows;
# the +1 on the last window covers the bias column (od 2304)
TF_WINS = [(0, 504), (504, 504), (1008, 504), (1512, 504), (2016, 289)]
OUT_SCALE = 1.0 / (S_W * S_I)  # out DRAM tensor is scaled by S_W*S_I


def _build_program(reps=1, ablate=frozenset(), internal=False):
    import concourse.bacc as bacc
    import concourse.tile as tile
    from concourse import mybir

    f32 = mybir.dt.float32
    f16 = mybir.dt.float16

    nc = bacc.Bacc("TRN2", target_bir_lowering=False, debug=False)

    kind = "Internal" if internal else "ExternalInput"
    okind = "Internal" if internal else "ExternalOutput"
    vis = nc.dram_tensor("vision", [C, HW], f32, kind=kind).ap()
    txt = nc.dram_tensor("text", [NC, KD], f32, kind=kind).ap()
    wtT = nc.dram_tensor("wtT", [KD, OD], f16, kind=kind).ap()
    btd = nc.dram_tensor("bt", [OD], f16, kind=kind).ap()
    out = nc.dram_tensor("out", [NC, HW], f16, kind=okind).ap()
    s_dram = nc.dram_tensor("s_scratch", [NWIN, WS], f16, kind="Internal").ap()
    zer = nc.dram_tensor("zeros_pw", [128, PW], f16, kind="ExternalInput").ap()
    bl_dram = nc.dram_tensor("bl_scratch", [NWIN, WS], f16, kind="Internal").ap()
    if internal:
        dummy = nc.dram_tensor("bench_in", [1, 8], f32, kind="ExternalInput").ap()
        dout = nc.dram_tensor("bench_out", [1, 8], f32, kind="ExternalOutput").ap()
    aps = (vis, txt, wtT, btd, out, s_dram, bl_dram, zer)

    with tile.TileContext(nc) as tc:
        if internal:
            with tc.tile_pool(name="dummyp", bufs=1) as dp:
                dtile = dp.tile([1, 8], f32)
                nc.sync.dma_start(out=dtile, in_=dummy)
                nc.sync.dma_start(out=dout, in_=dtile)
        if reps == 1:
            _emit_iteration(nc, tc, mybir, aps, ablate)
        else:
            with tc.For_i(0, reps, 1):
                _emit_iteration(nc, tc, mybir, aps, ablate)

    nc.compile()
    return nc


def _emit_iteration(nc, tc, mybir, aps, ablate=frozenset()):
    from contextlib import ExitStack
    from concourse.masks import make_identity

    vis, txt, wtT, btd, out, s_dram, bl_dram, zer = aps
    f32 = mybir.dt.float32
    f32r = mybir.dt.float32r
    f16 = mybir.dt.float16
    f8 = mybir.dt.float8e4
    DR = mybir.MatmulPerfMode.DoubleRow
    MUL = mybir.AluOpType.mult
    ADD = mybir.AluOpType.add
    AX = mybir.ActivationFunctionType
    X = mybir.AxisListType.X

    with ExitStack() as ctx:
        singles = ctx.enter_context(tc.tile_pool(name="singles", bufs=1))
        persist = ctx.enter_context(tc.tile_pool(name="persist", bufs=1))
        rawp = ctx.enter_context(tc.tile_pool(name="rawp", bufs=5))
        sqp = ctx.enter_context(tc.tile_pool(name="sqp", bufs=3))
        nrmp = ctx.enter_context(tc.tile_pool(name="nrmp", bufs=3))
        stgp = ctx.enter_context(tc.tile_pool(name="stgp", bufs=3))
        blp = ctx.enter_context(tc.tile_pool(name="blp", bufs=3))
        blinep = ctx.enter_context(tc.tile_pool(name="blinep", bufs=3))
        blsbp = ctx.enter_context(tc.tile_pool(name="blsbp", bufs=4))
        outp = ctx.enter_context(tc.tile_pool(name="outp", bufs=3))
        psnrm = ctx.enter_context(tc.tile_pool(name="psnrm", bufs=2, space="PSUM"))
        pss = ctx.enter_context(tc.tile_pool(name="pss", bufs=2, space="PSUM"))

        # ---------------- constants ----------------
        ident32 = singles.tile([128, 128], f32)
        make_identity(nc, ident32)
        ident16 = singles.tile([128, 128], f16)
        make_identity(nc, ident16)
        ones32f = singles.tile([128, 128], f32)
        nc.gpsimd.memset(ones32f, 1.0)
        ones32 = singles.tile([128, 128], f32r)
        nc.scalar.copy(ones32, ones32f)
        ones8 = singles.tile([128, 2, 128], f8)
        nc.gpsimd.memset(ones8, 1.0)
        ones16 = singles.tile([1, 128], f16)
        nc.gpsimd.memset(ones16, 1.0)
        # tridiagonal vertical-sum selector, scaled by S_W/9
        tri = singles.tile([128, 128], f16)
        nc.gpsimd.memset(tri, 0.0)
        for cm, pat, base in (() if "notri" in ablate else ((1, -1, 0), (-1, 1, 1), (1, -1, 1))):
            # zero set of cm*x + pat*y + base selects the band; bases >= 0
            nc.gpsimd.affine_select(
                out=tri, in_=tri,
                compare_op=mybir.AluOpType.not_equal,
                fill=S_W / 9.0, base=base,
                pattern=[[pat, 128]], channel_multiplier=cm,
            )
        sel4 = singles.tile([ROWS_PER_WIN, ROWS_PER_WIN, W], f16)
        nc.gpsimd.memset(sel4, 0.0)
        nc.gpsimd.affine_select(
            out=sel4, in_=sel4,
            compare_op=mybir.AluOpType.not_equal,
            fill=1.0, base=0,
            pattern=[[-1, ROWS_PER_WIN], [0, W]], channel_multiplier=1,
        )
        zrow = singles.tile([1, 256], f16)
        nc.gpsimd.memset(zrow, 0.0)
        bt16 = singles.tile([1, OD], f16)
        nc.sync.dma_start(out=bt16, in_=btd.unsqueeze(0))


        # ---------------- persistent tiles ----------------
        pad8 = persist.tile([128, CCH, PH, PWA], f8, name="pad8")
        s_padT = persist.tile([128, PW], f16, name="s_padT")
        # zero-fill via per-row DMAs: same size class as the s-row DMAs so
        # FIFO dispatch order matches completion order (one big DMA or a
        # gpsimd memset completes late and wipes freshly-written rows)
        for p in range(128):
            nc.sync.dma_start(out=s_padT[p:p + 1, :], in_=zer[p:p + 1, :])
        # dual-fp8 ldweights: k-tile stride must be 0 mod 16 elements
        w8s = [
            persist.tile([128, KK, CCH, cnt], f8, name=f"w8_{i}")
            for i, cnt in enumerate([NC0, 32])
        ]
        t_hatT = persist.tile([128, KCH, NC], f16)
        tfs = [persist.tile([128, OD], f32, name=f"tf{i}") for i in range(2)]
        bias_sc = persist.tile([128, 2], f32, name="bias_sc")

        # zero borders (halo) of pad8
        nc.gpsimd.memset(pad8[:, :, 0, :], 0.0)
        nc.gpsimd.memset(pad8[:, :, PH - 1, :], 0.0)
        nc.gpsimd.memset(pad8[:, :, :, 0], 0.0)
        nc.gpsimd.memset(pad8[:, :, :, PW - 1], 0.0)

        # ---------------- phase A: text -> conv weights ----------------
        def emit_phase_a(between=None):
            with (
                tc.tile_pool(name="pa", bufs=1) as pa,
                tc.tile_pool(name="pawt", bufs=10) as pawt,
                tc.tile_pool(name="pst", bufs=2, space="PSUM") as pstp,
                tc.tile_pool(name="psw", bufs=1, space="PSUM") as pswp,
            ):
                t_sbs = []
                for i, (n0, cnt) in enumerate([(0, NC0), (NC0, NC1)]):
                    t_sb = pa.tile([128, KD], f32, name=f"t_sb{i}")
                    nc.sync.dma_start(out=t_sb[:cnt], in_=txt[n0:n0 + cnt])
                    t_sbs.append(t_sb)

                tsq = pa.tile([128, KD], f32)
                stat = pa.tile([128, 4], f32)
                for i, (n0, cnt) in enumerate([(0, NC0), (NC0, NC1)]):
                    t_sb = t_sbs[i]
                    nc.scalar.activation(
                        out=tsq[:cnt], in_=t_sb[:cnt], func=AX.Square,
                        accum_out=stat[:cnt, 0:1],
                    )
                    nc.scalar.sqrt(stat[:cnt, 1:2], stat[:cnt, 0:1])
                    nc.vector.reciprocal(stat[:cnt, 2:3], stat[:cnt, 1:2])
                    nc.vector.tensor_scalar_mul(t_sb[:cnt], t_sb[:cnt], stat[:cnt, 2:3])
                    for k in range(KCH):
                        pst = pstp.tile([128, 128], f32, name="pst", tag="pst")
                        nc.tensor.transpose(
                            pst[:, :cnt], t_sb[:cnt, k * 128:(k + 1) * 128],
                            ident32[:cnt, :cnt],
                        )
                        nc.scalar.copy(t_hatT[:, k, n0:n0 + cnt], pst[:, :cnt])

                if between is not None:
                    between()

                # tf = t_hat @ Wt.T + bt  (fp16 in, fp32 accum; Wt^T slices
                # streamed from DRAM, shared by both nc chunks), with the
                # softmax (exp, group-sum, normalize) pipelined per window
                chunks = [(0, 0, NC0), (1, NC0, NC1)]
                w16s = [
                    pa.tile([128, C * KK], f16, name=f"w16_{i}", tag=f"w16_{i}")
                    for i in range(2)
                ]

                def emit_wtrans(i, cc):
                    n0, cnt = [(0, NC0), (NC0, NC1)][i]
                    w16v = w16s[i].rearrange("p (c t) -> p c t", t=KK)
                    for tap in range(KK):
                        pst = pstp.tile([128, 128], f16, name="pst", tag="pst")
                        nc.tensor.transpose(
                            pst[:, :cnt],
                            w16v[:cnt, cc * 128:(cc + 1) * 128, tap],
                            ident16[:cnt, :cnt],
                        )
                        # dw8 = (w - 1/9) * S_W, fp8
                        nc.scalar.activation(
                            out=w8s[i][:, tap, cc, :cnt], in_=pst[:, :cnt],
                            func=AX.Copy, scale=S_W, bias=-S_W / 9.0,
                        )

                for wi, (o0, ws) in enumerate(TF_WINS):
                    psws = [
                        pswp.tile([128, 512], f32, tag=f"psw{i}", name=f"psw{i}")
                        for i in range(2)
                    ]
                    for k in range(KCH):
                        wtsl = pawt.tile([128, 512], f16, tag="wtsl")
                        nc.sync.dma_start(
                            out=wtsl[:, :ws],
                            in_=wtT[k * 128:(k + 1) * 128, o0:o0 + ws],
                        )
                        for i, n0, cnt in chunks:
                            nc.tensor.matmul(
                                psws[i][:cnt, :ws],
                                t_hatT[:, k, n0:n0 + cnt],
                                wtsl[:, :ws],
                                start=(k == 0),
                                stop=False,
                            )
                    wse = ws - 1 if o0 + ws > C * KK else ws  # exp excl. bias col
                    ngrp = wse // KK
                    for i, n0, cnt in chunks:
                        nc.tensor.matmul(
                            psws[i][:cnt, :ws],
                            ones16[:1, :cnt],
                            bt16[:1, o0:o0 + ws],
                            start=False,
                            stop=True,
                        )
                        nc.scalar.copy(tfs[i][:cnt, o0:o0 + ws], psws[i][:cnt, :ws])
                        tfw = tfs[i][:cnt, o0:o0 + wse]
                        nc.scalar.activation(out=tfw, in_=tfw, func=AX.Exp)
                        tfv = tfw.rearrange("p (c t) -> p c t", t=KK)
                        ssum = pa.tile([128, C], f32, tag=f"ssum{i}", name="ssum")
                        nc.vector.reduce_sum(out=ssum[:cnt, :ngrp], in_=tfv, axis=X)
                        rsum = pa.tile([128, C], f32, tag=f"rsum{i}", name="rsum")
                        nc.vector.reciprocal(rsum[:cnt, :ngrp], ssum[:cnt, :ngrp])
                        nc.vector.tensor_tensor(
                            out=w16s[i][:cnt, o0:o0 + wse].rearrange(
                                "p (c t) -> p c t", t=KK
                            ),
                            in0=tfv,
                            in1=rsum[:cnt, :ngrp].unsqueeze(2).broadcast_to(
                                [cnt, ngrp, KK]
                            ),
                            op=MUL,
                        )
                        # weight transposes as soon as their c-block columns
                        # are done: cc=0 spans tf windows 0-2, cc=1 spans 2-4
                        if wi == 2:
                            emit_wtrans(i, 0)
                        elif wi == 4:
                            emit_wtrans(i, 1)
                for i, n0, cnt in chunks:
                    # conv bias prescaled by S_W*S_I for the scaled eviction
                    nc.scalar.activation(
                        out=bias_sc[:cnt, i:i + 1],
                        in_=tfs[i][:cnt, C * KK:C * KK + 1],
                        func=AX.Copy, scale=S_W * S_I,
                    )

        # -------- vision pipeline (software-pipelined emission) --------
        state = {}

        def front(w):
            raws = []
            ps_n = psnrm.tile([128, WS], f32, tag="nrm", name="ps_n")
            ps_s = pss.tile([128, WS], f32, tag="s", name="ps_s")
            sq8 = sqp.tile([128, CCH, WS], f8, tag="sq8", name="sq8")
            for cc in range(CCH):
                raw = rawp.tile([128, WS], f32r, name=f"raw{cc}", tag=f"raw{cc}")
                nc.sync.dma_start(
                    out=raw,
                    in_=vis[cc * 128:(cc + 1) * 128, w * WS:(w + 1) * WS].bitcast(f32r),
                )
                raws.append(raw)
            for cc in range(CCH):
                nc.gpsimd.tensor_tensor(
                    out=sq8[:, cc, :], in0=raws[cc], in1=raws[cc], op=MUL
                )
                nc.tensor.matmul(
                    ps_s, ones32, raws[cc],
                    start=(cc == 0), stop=(cc == CCH - 1),
                )
            nc.tensor.matmul(ps_n, ones8, sq8, perf_mode=DR, start=True, stop=True)
            state[w] = [raws, ps_n, ps_s, None]

        def mid(w):
            r0 = w * ROWS_PER_WIN
            inv = nrmp.tile([128, WS], f32, tag="inv", name="inv")
            nc.scalar.activation(
                out=inv, in_=state[w][1], func=AX.Abs_reciprocal_sqrt,
                scale=1.0 / (S_I * S_I),
            )
            state[w][3] = inv
            # s rows (x S_I, fp16): staged broadcast product, then row-wise
            # SBUF->SBUF DMAs redistribute rows onto partitions. stgp has one
            # buffer per window: DMA-involved WAR deps are unreliable, so
            # rotation must never catch up with pending reads.
            s_stage = stgp.tile([128, ROWS_PER_WIN, W], f16, tag="sst", name="sst")
            nc.vector.tensor_tensor(
                out=s_stage,
                in0=state[w][2].rearrange("p (r x) -> p r x", x=W),
                in1=inv.rearrange("p (r x) -> p r x", x=W),
                op=MUL,
            )
            for r in range(ROWS_PER_WIN):
                nc.sync.dma_start(
                    out=s_padT[r0 + r:r0 + r + 1, 1:1 + W],
                    in_=s_stage[0:1, r, :],
                )

        def back(w):
            raws, _, _, inv = state.pop(w)
            r0 = w * ROWS_PER_WIN
            inv_v = inv.rearrange("p (r x) -> p r x", x=W)
            for cc in range(CCH):
                nc.vector.tensor_tensor(
                    out=pad8[:, cc, 1 + r0:1 + r0 + ROWS_PER_WIN, 1:1 + W],
                    in0=raws[cc].rearrange("p (r x) -> p r x", x=W),
                    in1=inv_v,
                    op=MUL,
                )

        def bstage(w, psv):
            # vertical 3-sum via tridiagonal matmul; rows outside the image
            # clip via the band structure of tri
            ps_v = psv.tile([ROWS_PER_WIN, PW], f32, tag="v", name="ps_v")
            nc.tensor.matmul(
                ps_v, tri[:, w * ROWS_PER_WIN:(w + 1) * ROWS_PER_WIN], s_padT,
                start=True, stop=True,
            )
            v_sb = blp.tile([ROWS_PER_WIN, PW], f16, tag="v_sb", name="v_sb")
            nc.scalar.copy(v_sb, ps_v)
            h1 = blp.tile([ROWS_PER_WIN, PW - 1], f16, tag="h1", name="h1")
            nc.vector.tensor_tensor(
                out=h1, in0=v_sb[:, 0:PW - 1], in1=v_sb[:, 1:PW], op=ADD
            )
            bline = blinep.tile([ROWS_PER_WIN, W], f16, tag="bline", name="bline")
            nc.vector.tensor_tensor(
                out=bline, in0=h1[:, 0:W], in1=v_sb[:, 2:PW], op=ADD
            )
            return bline

        def conv(w, psc0, psc1, bline):
            noinject = "noinject" in ablate
            r0 = w * ROWS_PER_WIN
            for i, n0, cnt in [(0, 0, NC0), (1, NC0, NC1)]:
                pool = psc0 if i == 0 else psc1
                ps = pool.tile([128, WS], f32, tag=f"c{i}", name=f"c{i}")
                for tap in range(KK):
                    ty, tx = tap // 3, tap % 3
                    nc.tensor.matmul(
                        ps[:cnt],
                        w8s[i][:, tap, :, :cnt],
                        pad8[:, :, r0 + ty:r0 + ty + ROWS_PER_WIN, tx:tx + W],
                        start=(tap == 0),
                        stop=(noinject and tap == KK - 1), perf_mode=DR,
                    )
                if not noinject:
                    # base inject: per-row selector matmuls, K=4, no DMA
                    for r in range(ROWS_PER_WIN):
                        nc.tensor.matmul(
                            ps[:cnt, r * W:(r + 1) * W],
                            sel4[:, r, :cnt],
                            bline,
                            start=False, stop=(r == ROWS_PER_WIN - 1),
                        )
                osb = outp.tile([128, WS], f16, tag=f"osb{i}", name=f"osb{i}")
                nc.vector.tensor_scalar_add(
                    osb[:cnt], ps[:cnt], bias_sc[:cnt, i:i + 1]
                )
                nc.sync.dma_start(
                    out=out[n0:n0 + cnt, w * WS:(w + 1) * WS], in_=osb[:cnt]
                )

        cursor = [0]

        def step_norm():
            w = cursor[0]
            if w < NWIN:
                front(w)
            if 1 <= w < NWIN + 1:
                mid(w - 1)
            if 2 <= w < NWIN + 2:
                back(w - 2)
            cursor[0] += 1

        def ensure_norm(upto):
            while cursor[0] <= min(upto, NWIN + 1):
                step_norm()

        def _prologue():
            pass  # no burst: conv loop paces all pools at steady rhythm

        if "phasea" not in ablate:
            emit_phase_a(between=_prologue)
        else:
            _prologue()

        if "conv" in ablate:
            ensure_norm(NWIN + 1)
            return

        with (
            tc.tile_pool(name="psc0", bufs=2, space="PSUM") as psc0,
            tc.tile_pool(name="psc1", bufs=1, space="PSUM") as psc1,
            tc.tile_pool(name="psv", bufs=1, space="PSUM") as psv,
        ):
            noinj = "noinject" in ablate
            blines = {}
            for w in range(NWIN):
                ensure_norm(w + 3)
                if not noinj:
                    blines[w] = bstage(w, psv)
                conv(w, psc0, psc1, blines.pop(w, None))


_NC_CACHE = {}


def _get_program(reps=1, ablate=frozenset(), internal=False):
    ablate = frozenset(ablate)
    key = (reps, ablate, internal)
    if key not in _NC_CACHE:
        _NC_CACHE[key] = _build_program(reps, ablate, internal)
    return _NC_CACHE[key]


def _make_in_maps(vision, text, Wt, bt):
    wtT16 = np.ascontiguousarray(Wt.astype(np.float32).T).astype(np.float16)
    bt16 = bt.astype(np.float16)
    in_maps = []
    for b in range(B):
        in_maps.append({
            "vision": np.ascontiguousarray(vision[b].reshape(C, HW)),
            "text": np.ascontiguousarray(text[b, :, 0, :]),
            "wtT": wtT16,
            "bt": bt16,
            "zeros_pw": np.zeros((128, PW), np.float16),
        })
    return in_maps


def _run(vision, text, Wt, bt, trace=False):
    from concourse.bass_utils import run_bass_kernel_spmd

    nc = _get_program()
    in_maps = _make_in_maps(vision, text, Wt, bt)
    res = run_bass_kernel_spmd(nc, in_maps, list(range(B)), trace=trace)
    outs = np.stack([
        np.asarray(res.results[b]["out"]).astype(np.float32).reshape(NC, H, W)
        for b in range(B)
    ]) * OUT_SCALE
    return outs, res


def kernel(vision, text, Wt, bt):
    outs, _ = _run(vision, text, Wt, bt, trace=False)
    return outs
